# revision 6
# baseline (speedup 1.0000x reference)
"""Trainium2 Bass kernel for nn_CausalSelfAttention_12283606468211.

Sliding-window causal GQA attention (B=4, T=2048, C=1024, 16 q-heads,
4 kv-heads, head_dim 64, window 1024) with value-embedding gating,
RoPE + QK-RMSNorm, and output projection.

Sharding: 8 cores = 4 batches x 2 head-halves.  Core (2b+p) computes
q-heads [8p, 8p+8) and kv-heads [2p, 2p+2) of batch b for ALL 2048
queries, then all-gathers the per-head attention outputs within each
batch pair and computes its half of the output columns of the final
projection.  The SPMD program is identical on every core; all per-core
differences are carried by host-side input slicing.

All matmuls run in bf16 (inputs pre-cast on host); accumulation is
fp32 in PSUM.  Softmax needs no max-subtraction because QK-RMSNorm
bounds |score| <= 8.  The k-side RMSNorm scale folds into the
per-partition `scale` operand of the Exp activation; the softmax
denominator comes free from an appended ones-column in the PV matmul.
Causal/window masking is done with small constant-matrix matmuls that
accumulate -1e9 triangles straight into the score PSUM.
"""

import sys

sys.path.insert(0, "/opt/trn_rl_repo")

import numpy as np
import ml_dtypes

import concourse.bass as bass
import concourse.mybir as mybir
from concourse import bacc
from concourse.tile import TileContext
from concourse.bass_utils import run_bass_kernel_spmd
from concourse.masks import make_causal_mask

F32 = mybir.dt.float32
BF16 = mybir.dt.bfloat16
AF = mybir.ActivationFunctionType
ALU = mybir.AluOpType

# Problem constants
B, T, C = 4, 2048, 1024
WINDOW = 1024
EPS = 1.1920928955078125e-07

# Per-core constants (head-split SPMD)
NH = 8          # own q-heads
NKV = 2         # own kv-heads
D = 64
QDIM = NH * D   # 512
KVDIM = NKV * D  # 128
NT = T // 128   # 16 token tiles
NG = T // 256   # 8 query groups of 256
VSLOT = D + 1   # 65: [v(64) | one] per kv head

_CACHE = {}


def _union_chunks(g):
    lo = max(0, 2 * g - 8)
    hi = min(NT - 1, 2 * g + 1)
    return list(range(lo, hi + 1))


def _qsel(c, g):
    """Column range [c0, c1) within group g covered by key chunk c."""
    lo_t = max(2 * g, c)
    hi_t = min(2 * g + 1, c + 8)
    if lo_t > hi_t:
        return None
    return ((lo_t - 2 * g) * 128, (hi_t + 1 - 2 * g) * 128)


def build_program():
    nc = bacc.Bacc("TRN2", target_bir_lowering=False, debug=False,
                   enable_asserts=True, num_devices=8)

    x_d = nc.dram_tensor("x_bf", [T, C], BF16, kind="ExternalInput")
    ve_d = nc.dram_tensor("ve_bf", [T, KVDIM], BF16, kind="ExternalInput")
    wq_d = nc.dram_tensor("wq_bf", [C, QDIM], BF16, kind="ExternalInput")
    wk_d = nc.dram_tensor("wk_bf", [C, KVDIM], BF16, kind="ExternalInput")
    wv_d = nc.dram_tensor("wv_bf", [C, KVDIM], BF16, kind="ExternalInput")
    wg_d = nc.dram_tensor("wg_bf", [8, NKV], BF16, kind="ExternalInput")
    wp_d = nc.dram_tensor("wp_bf", [C, 512], BF16, kind="ExternalInput")
    c2_d = nc.dram_tensor("c2_bf", [T, D], BF16, kind="ExternalInput")   # [cos|cos]
    s2_d = nc.dram_tensor("s2_bf", [T, D], BF16, kind="ExternalInput")   # [sin|-sin]
    y_d = nc.dram_tensor("y_out", [T, 512], F32, kind="ExternalOutput")

    with TileContext(nc) as tc:
        with (
            tc.tile_pool(name="const", bufs=1) as constp,
            tc.tile_pool(name="persist", bufs=1) as pers,
        ):
            # ---- constants ----
            ident = constp.tile([128, 128], BF16, name="ident")
            nc.gpsimd.memset(ident[:, :], 0.0)
            nc.gpsimd.affine_select(
                out=ident[:, :], in_=ident[:, :],
                compare_op=ALU.not_equal, fill=1.0, base=0,
                pattern=[[-1, 128]], channel_multiplier=1)
            # diag mask (as matmul lhsT): psum[tk,tq] += M[tq,tk]; want -1e9
            # where tk > tq -> M strict upper triangular = causal mask.
            mdiag = constp.tile([128, 128], BF16, name="mdiag")
            make_causal_mask(nc, mdiag[:, :], mask_val=-1.0e9)
            # far mask: -1e9 where tk <= tq -> M lower (incl. diag)
            b64e = constp.tile([128, 1], F32, name="b64e")
            nc.gpsimd.memset(b64e[:, :], 64.0 * EPS)
            beps = constp.tile([128, 1], F32, name="beps")
            nc.gpsimd.memset(beps[:, :], EPS)
            mfar = constp.tile([128, 128], BF16, name="mfar")
            nc.gpsimd.memset(mfar[:, :], 0.0)
            nc.gpsimd.affine_select(
                out=mfar[:, :], in_=mfar[:, :],
                compare_op=ALU.is_ge, fill=-1.0e9, base=-1,
                pattern=[[1, 128]], channel_multiplier=-1)

            # ---- weights / rotary tables ----
            wq_sb = pers.tile([128, 8, QDIM], BF16, name="wq_sb")
            wk_sb = pers.tile([128, 8, KVDIM], BF16, name="wk_sb")
            wv_sb = pers.tile([128, 8, KVDIM], BF16, name="wv_sb")
            wp_sb = pers.tile([128, 8, 512], BF16, name="wp_sb")
            wg_sb = pers.tile([8, NKV], BF16, name="wg_sb")
            c2_sb = pers.tile([128, NT, D], BF16, name="c2_sb")
            s2_sb = pers.tile([128, NT, D], BF16, name="s2_sb")
            for cc in range(8):
                sl = slice(cc * 128, (cc + 1) * 128)
                nc.sync.dma_start(wq_sb[:, cc], wq_d.ap()[sl, :])
                nc.sync.dma_start(wk_sb[:, cc], wk_d.ap()[sl, :])
                nc.sync.dma_start(wv_sb[:, cc], wv_d.ap()[sl, :])
                nc.sync.dma_start(wp_sb[:, cc], wp_d.ap()[sl, :])
            nc.sync.dma_start(wg_sb[:, :], wg_d.ap())
            nc.sync.dma_start(c2_sb[:, :, :],
                              c2_d.ap().rearrange("(t p) d -> p t d", p=128))
            nc.sync.dma_start(s2_sb[:, :, :],
                              s2_d.ap().rearrange("(t p) d -> p t d", p=128))

            # ---- xT via DMA transpose ----
            xT = [pers.tile([128, 4, 512], BF16, name=f"xT{cc}") for cc in range(8)]
            for tg in range(4):
                for cc in range(8):
                    nc.sync.dma_start_transpose(
                        out=xT[cc][:, tg],
                        in_=x_d.ap()[tg * 512:(tg + 1) * 512,
                                     cc * 128:(cc + 1) * 128])

            # persistent attention operands
            qT = [pers.tile([128, T], BF16, name=f"qT{jp}") for jp in range(4)]
            kT = [pers.tile([128, 128], BF16, name=f"kTn{c}") for c in range(NT)]
            kTs = [pers.tile([128, 128], BF16, name=f"kTs{c}") for c in range(NT)]
            v_sb = [pers.tile([128, NKV * VSLOT], BF16, name=f"v{c}")
                    for c in range(NT)]
            rs_k = [pers.tile([128, NKV], F32, name=f"rsk{c}") for c in range(NT)]
            yT_all = [pers.tile([128, T], BF16, name=f"yTa{m}") for m in range(4)]

            # ================= phase 1: qkv / gate / rope / norms ========
            with (
                tc.tile_pool(name="work", bufs=3) as work,
                tc.tile_pool(name="qps", bufs=2, space="PSUM") as qpsp,
                tc.tile_pool(name="kps", bufs=2, space="PSUM") as kpsp,
                tc.tile_pool(name="vps", bufs=2, space="PSUM") as vpsp,
                tc.tile_pool(name="zps", bufs=2, space="PSUM") as zpsp,
            ):
                for t in range(NT):
                    tg, to = t // 4, (t % 4) * 128
                    q_ps = qpsp.tile([128, QDIM], F32, tag="q_ps")
                    k_ps = kpsp.tile([128, KVDIM], F32, tag="k_ps")
                    v_ps = vpsp.tile([128, KVDIM], F32, tag="v_ps")
                    z_ps = zpsp.tile([128, NKV], F32, tag="z_ps")
                    for cc in range(8):
                        lhs = xT[cc][:, tg, to:to + 128]
                        st, sp_ = (cc == 0), (cc == 7)
                        nc.tensor.matmul(q_ps[:, :], lhs, wq_sb[:, cc],
                                         start=st, stop=sp_)
                        nc.tensor.matmul(k_ps[:, :], lhs, wk_sb[:, cc],
                                         start=st, stop=sp_)
                        nc.tensor.matmul(v_ps[:, :], lhs, wv_sb[:, cc],
                                         start=st, stop=sp_)
                    nc.tensor.matmul(z_ps[:, :], xT[0][0:8, tg, to:to + 128],
                                     wg_sb[:, :], start=True, stop=True)

                    # gate = 2*sigmoid(z) = 2/(1+exp(-z))
                    e_sb = work.tile([128, NKV], F32, tag="e_sb")
                    nc.scalar.activation(e_sb[:, :], z_ps[:, :], AF.Exp,
                                         bias=0.0, scale=-1.0)
                    gp1 = work.tile([128, NKV], F32, tag="gp1")
                    nc.vector.tensor_scalar_add(gp1[:, :], e_sb[:, :], 1.0)
                    gr = work.tile([128, NKV], F32, tag="gr")
                    nc.vector.reciprocal(gr[:, :], gp1[:, :])

                    # v' = v + 2*gate*ve  into [v|one] slots
                    ve_t = work.tile([128, KVDIM], BF16, tag="ve_t")
                    nc.sync.dma_start(ve_t[:, :],
                                      ve_d.ap()[t * 128:(t + 1) * 128, :])
                    ves = work.tile([128, KVDIM], BF16, tag="ves")
                    for u in range(NKV):
                        nc.vector.tensor_scalar(
                            ves[:, u * D:(u + 1) * D], ve_t[:, u * D:(u + 1) * D],
                            gr[:, u:u + 1], 2.0, ALU.mult, ALU.mult)
                    vv = v_sb[t][:, :].rearrange("p (u s) -> p u s", u=NKV)
                    nc.vector.tensor_tensor(
                        vv[:, :, 0:D],
                        v_ps[:, :].rearrange("p (u d) -> p u d", u=NKV),
                        ves[:, :].rearrange("p (u d) -> p u d", u=NKV),
                        ALU.add)
                    nc.gpsimd.memset(vv[:, :, D:D + 1], 1.0)

                    # ---- q path ----
                    q_sb = work.tile([128, QDIM], BF16, tag="q_sb")
                    nc.vector.tensor_copy(q_sb[:, :], q_ps[:, :])
                    sq = work.tile([128, QDIM], BF16, tag="sq")
                    nc.vector.tensor_mul(sq[:, :], q_sb[:, :], q_sb[:, :])
                    msum = work.tile([128, NH], F32, tag="msum")
                    nc.vector.tensor_reduce(
                        msum[:, :], sq[:, :].rearrange("p (h d) -> p h d", h=NH),
                        mybir.AxisListType.X, ALU.add)
                    # rs_q = rsqrt(sum(q^2)+64eps) = rsqrt(mean+eps)/8
                    lnq = work.tile([128, NH], F32, tag="lnq")
                    nc.scalar.activation(lnq[:, :], msum[:, :], AF.Ln,
                                         bias=b64e[:, :], scale=1.0)
                    rs_q = work.tile([128, NH], F32, tag="rs_q")
                    nc.scalar.activation(rs_q[:, :], lnq[:, :], AF.Exp,
                                         bias=0.0, scale=-0.5)

                    # rope(q): out1 = q1*c + q2*s ; out2 = q2*c - q1*s
                    qg = q_sb[:, :].rearrange("p (h s f) -> p h s f", h=NH, s=2)
                    c2b = c2_sb[:, t].unsqueeze(1).broadcast_to([128, NH, D])
                    s2b = s2_sb[:, t].rearrange("p (s f) -> p s f", s=2)
                    m1 = work.tile([128, QDIM], BF16, tag="m1")
                    m1g = m1[:, :].rearrange("p (h s f) -> p h s f", h=NH, s=2)
                    nc.vector.tensor_mul(
                        m1g[:, :, 0], qg[:, :, 1],
                        s2b[:, 0].unsqueeze(1).broadcast_to([128, NH, 32]))
                    nc.vector.tensor_mul(
                        m1g[:, :, 1], qg[:, :, 0],
                        s2b[:, 1].unsqueeze(1).broadcast_to([128, NH, 32]))
                    rq = work.tile([128, QDIM], BF16, tag="rq")
                    nc.vector.tensor_mul(
                        rq[:, :].rearrange("p (h d) -> p h d", h=NH),
                        q_sb[:, :].rearrange("p (h d) -> p h d", h=NH), c2b)
                    nc.vector.tensor_add(rq[:, :], rq[:, :], m1[:, :])
                    # q~ = rope(q) * rs_q (per head, offloaded to gpsimd)
                    qn = work.tile([128, QDIM], BF16, tag="qn")
                    for j in range(NH):
                        nc.gpsimd.tensor_scalar_mul(
                            qn[:, j * D:(j + 1) * D], rq[:, j * D:(j + 1) * D],
                            rs_q[:, j:j + 1])
                    for jp in range(4):
                        nc.sync.dma_start_transpose(
                            out=qT[jp][:, t * 128:(t + 1) * 128],
                            in_=qn[:, jp * 128:(jp + 1) * 128])

                    # ---- k path ----
                    k_sb = work.tile([128, KVDIM], BF16, tag="k_sb")
                    nc.vector.tensor_copy(k_sb[:, :], k_ps[:, :])
                    sqk = work.tile([128, KVDIM], BF16, tag="sqk")
                    nc.vector.tensor_mul(sqk[:, :], k_sb[:, :], k_sb[:, :])
                    ksum = work.tile([128, NKV], F32, tag="ksum")
                    nc.vector.tensor_reduce(
                        ksum[:, :], sqk[:, :].rearrange("p (u d) -> p u d", u=NKV),
                        mybir.AxisListType.X, ALU.add)
                    lnk = work.tile([128, NKV], F32, tag="lnk")
                    nc.scalar.activation(lnk[:, :], ksum[:, :], AF.Ln,
                                         bias=beps[:, :], scale=1.0 / 64.0)
                    nc.scalar.activation(rs_k[t][:, :], lnk[:, :], AF.Exp,
                                         bias=0.0, scale=-0.5)

                    kg = k_sb[:, :].rearrange("p (u s f) -> p u s f", u=NKV, s=2)
                    km = work.tile([128, KVDIM], BF16, tag="km")
                    kmg = km[:, :].rearrange("p (u s f) -> p u s f", u=NKV, s=2)
                    nc.vector.tensor_mul(
                        kmg[:, :, 0], kg[:, :, 1],
                        s2b[:, 0].unsqueeze(1).broadcast_to([128, NKV, 32]))
                    nc.vector.tensor_mul(
                        kmg[:, :, 1], kg[:, :, 0],
                        s2b[:, 1].unsqueeze(1).broadcast_to([128, NKV, 32]))
                    rk = work.tile([128, KVDIM], BF16, tag="rk")
                    nc.vector.tensor_mul(
                        rk[:, :].rearrange("p (u d) -> p u d", u=NKV),
                        k_sb[:, :].rearrange("p (u d) -> p u d", u=NKV),
                        c2_sb[:, t].unsqueeze(1).broadcast_to([128, NKV, D]))
                    nc.vector.tensor_add(rk[:, :], rk[:, :], km[:, :])
                    # kT[t]: rows 0-63 = kv0 dims, 64-127 = kv1 dims
                    nc.sync.dma_start_transpose(out=kT[t][:, :], in_=rk[:, :])
                    # kTs[t]: partition-halves swapped (kv1 on top)
                    nc.sync.dma_start(kTs[t][0:64, :], kT[t][64:128, :])
                    nc.sync.dma_start(kTs[t][64:128, :], kT[t][0:64, :])

            # ================= phase 2: attention =========================
            with (
                tc.tile_pool(name="probs", bufs=14) as probp,
                tc.tile_pool(name="yup", bufs=10) as yup,
                tc.tile_pool(name="att2", bufs=4) as att2,
                tc.tile_pool(name="sump_p", bufs=2) as sumpp,
                tc.tile_pool(name="scps", bufs=3, space="PSUM") as scpsp,
                tc.tile_pool(name="ytps", bufs=3, space="PSUM") as ytpsp,
                tc.tile_pool(name="drams", bufs=1, space="DRAM") as dramsp,
            ):
                rdram = dramsp.tile([NG, 8, 256], F32, name="rdram")
                for g in range(NG):
                    cs = _union_chunks(g)
                    sump = sumpp.tile([8, 256], F32, tag="sump")
                    yu_list = []
                    for j in range(NH):
                        sp = j % 2
                        u = j // 4
                        jp = j // 2
                        base = sp * 64
                        kts = kT if u == sp else kTs
                        probs = []
                        for c in cs:
                            qs = _qsel(c, g)
                            c0, c1 = qs
                            sc = scpsp.tile([128, 256], F32, tag="sc")
                            has_mask = (c in (2 * g, 2 * g + 1)) or \
                                       (c + 8 in (2 * g, 2 * g + 1))
                            nc.tensor.matmul(
                                sc[:, c0:c1],
                                kts[c][base:base + 64, :],
                                qT[jp][base:base + 64,
                                       g * 256 + c0:g * 256 + c1],
                                start=True, stop=True)
                            if c in (2 * g, 2 * g + 1):      # diagonal chunk
                                d0 = (c - 2 * g) * 128
                                nc.tensor.matmul(sc[:, d0:d0 + 128],
                                                 mdiag[:, :], ident[:, :],
                                                 start=False, stop=True,
                                                 skip_group_check=True)
                            if c + 8 in (2 * g, 2 * g + 1):  # far chunk
                                f0 = (c + 8 - 2 * g) * 128
                                nc.tensor.matmul(sc[:, f0:f0 + 128],
                                                 mfar[:, :], ident[:, :],
                                                 start=False, stop=True,
                                                 skip_group_check=True)
                            pt = probp.tile([128, 256], BF16, tag="pt")
                            if c0 > 0:
                                nc.gpsimd.memset(pt[:, 0:c0], 0.0)
                            if c1 < 256:
                                nc.gpsimd.memset(pt[:, c1:256], 0.0)
                            nc.scalar.activation(pt[:, c0:c1], sc[:, c0:c1],
                                                 AF.Exp, bias=0.0,
                                                 scale=rs_k[c][:, u:u + 1])
                            probs.append(pt)
                        yt = ytpsp.tile([VSLOT, 256], F32, tag="yt")
                        for ci, c in enumerate(cs):
                            nc.tensor.matmul(
                                yt[:, :],
                                v_sb[c][:, u * VSLOT:(u + 1) * VSLOT],
                                probs[ci][:, :],
                                start=(ci == 0), stop=(ci == len(cs) - 1))
                        # unnormalized y + sum row out of PSUM
                        yu = yup.tile([VSLOT, 256], F32, tag="yu")
                        nc.vector.tensor_copy(yu[:, :], yt[:, :])
                        nc.sync.dma_start(sump[j:j + 1, :], yu[64:65, :])
                        yu_list.append(yu)
                    # batched reciprocal of the 8 softmax denominators
                    rec = sumpp.tile([8, 256], F32, tag="rec")
                    nc.vector.reciprocal(rec[:, :], sump[:, :])
                    nc.sync.dma_start(rdram[g], rec[:, :])
                    for j in range(NH):
                        rb = att2.tile([64, 256], F32, tag="rb")
                        nc.sync.dma_start(
                            rb[:, :],
                            rdram[g, j].unsqueeze(0).to_broadcast((64, 256)))
                        m = j // 2
                        if j % 2 == 0:
                            nc.vector.tensor_mul(
                                yT_all[m][0:64, g * 256:(g + 1) * 256],
                                yu_list[j][0:64, :], rb[:, :])
                        else:
                            yn = att2.tile([64, 256], BF16, tag="yn")
                            nc.vector.tensor_mul(yn[:, :], yu_list[j][0:64, :],
                                                 rb[:, :])
                            nc.sync.dma_start(
                                yT_all[m][64:128, g * 256:(g + 1) * 256],
                                yn[:, :])

            # ================= phase 3: exchange + projection =============
            with (
                tc.tile_pool(name="proj_sb", bufs=1) as projp,
                tc.tile_pool(name="proj_w", bufs=3) as projw,
                tc.tile_pool(name="prps", bufs=2, space="PSUM") as prpsp,
                tc.tile_pool(name="dram3", bufs=1, space="DRAM") as dram3p,
            ):
                agin = dram3p.tile([4, 128, T], BF16, name="agin")
                agout = dram3p.tile([8, 128, T], BF16, name="agout")
                for m in range(4):
                    nc.sync.dma_start(agin[m], yT_all[m][:, :])
                nc.gpsimd.collective_compute(
                    "AllGather", ALU.bypass,
                    replica_groups=[[0, 1], [2, 3], [4, 5], [6, 7]],
                    ins=[agin[:, :, :].opt()],
                    outs=[agout[:, :, :].opt()])
                yg = [projp.tile([128, T], BF16, name=f"yg{cc}") for cc in range(8)]
                for cc in range(8):
                    nc.sync.dma_start(yg[cc][:, :], agout[cc])
                for i in range(NT):
                    pr = prpsp.tile([128, 512], F32, tag="pr")
                    for cc in range(8):
                        nc.tensor.matmul(pr[:, :],
                                         yg[cc][:, i * 128:(i + 1) * 128],
                                         wp_sb[:, cc],
                                         start=(cc == 0), stop=(cc == 7))
                    o_sb = projw.tile([128, 512], F32, tag="o_sb")
                    nc.vector.tensor_copy(o_sb[:, :], pr[:, :])
                    nc.sync.dma_start(y_d.ap()[i * 128:(i + 1) * 128, :],
                                      o_sb[:, :])

    nc.compile()
    return nc


def _prep_inputs(x, ve, cos, sin, wq, wk, wv, wproj, wgate):
    bf = ml_dtypes.bfloat16
    cosf = np.asarray(cos, np.float32).reshape(T, 32)
    sinf = np.asarray(sin, np.float32).reshape(T, 32)
    c2 = np.concatenate([cosf, cosf], axis=1).astype(bf)
    s2 = np.concatenate([sinf, -sinf], axis=1).astype(bf)
    x = np.asarray(x, np.float32)
    ve = np.asarray(ve, np.float32)
    wq = np.asarray(wq, np.float32)
    wk = np.asarray(wk, np.float32)
    wv = np.asarray(wv, np.float32)
    wproj = np.asarray(wproj, np.float32)
    wgate = np.asarray(wgate, np.float32)
    maps = []
    for core in range(8):
        b, hp = core // 2, core % 2
        maps.append({
            "x_bf": x[b].astype(bf),
            "ve_bf": ve[b][:, hp * 128:(hp + 1) * 128].astype(bf),
            "wq_bf": wq[:, hp * 512:(hp + 1) * 512].astype(bf),
            "wk_bf": wk[:, hp * 128:(hp + 1) * 128].astype(bf),
            "wv_bf": wv[:, hp * 128:(hp + 1) * 128].astype(bf),
            "wg_bf": wgate[:, hp * 2:(hp + 1) * 2].astype(bf),
            "wp_bf": wproj[:, hp * 512:(hp + 1) * 512].astype(bf),
            "c2_bf": c2,
            "s2_bf": s2,
        })
    return maps


def kernel(x, ve, cos, sin, wq, wk, wv, wproj, wgate, window):
    assert int(window) == WINDOW
    if "nc" not in _CACHE:
        _CACHE["nc"] = build_program()
    nc = _CACHE["nc"]
    maps = _prep_inputs(x, ve, cos, sin, wq, wk, wv, wproj, wgate)
    res = run_bass_kernel_spmd(nc, maps, list(range(8))).results
    y = np.empty((B, T, C), np.float32)
    for core in range(8):
        b, hp = core // 2, core % 2
        y[b][:, hp * 512:(hp + 1) * 512] = res[core]["y_out"]
    return y


# revision 15
# speedup vs baseline: 1.3703x; 1.3703x over previous
"""Trainium2 Bass kernel for nn_CausalSelfAttention_12283606468211.

Sliding-window causal GQA attention (B=4, T=2048, C=1024, 16 q-heads,
4 kv-heads, head_dim 64, window 1024) with value-embedding gating,
RoPE + QK-RMSNorm, and output projection.

Sharding: 8 cores = 4 batches x 2 head-halves.  Core (2b+p) computes
q-heads [8p, 8p+8) and kv-heads [2p, 2p+2) of batch b for ALL 2048
queries.  The per-head attention outputs are exchanged between the two
cores of a batch with small pipelined AllGathers (one per 256-query
group) and each core computes its half of the output columns of the
final projection.  The SPMD program is identical on every core; all
per-core differences are carried by host-side input slicing.

All matmuls run in bf16 (inputs pre-cast on host); accumulation is
fp32 in PSUM.  Softmax needs no max-subtraction because QK-RMSNorm
bounds |score| <= 8.  Sum-of-squares for the RMSNorms run on ScalarE
(Square + accum_out, straight from PSUM); the reciprocal square roots
are computed on the vector engine (bit-trick seed + 2 Newton
iterations), so the only ScalarE table set ever loaded is Exp's.  The
RMSNorm scales fold into q~ and k~ before the QK^T matmuls, so Exp
runs with a constant scale and scores for up to 4 key chunks share one
wide Exp op.  The softmax denominator comes free from an appended
ones-column in the PV matmul; its reciprocal is broadcast across
partitions with a K=1 fp32r matmul.  Causal/window masking accumulates
-1e9 triangle matrices into the score PSUM via constant matmuls.
Query heads are slotted as (j%4, j//4) in the transposed tiles so each
head's partition half matches its kv-head's half in the transposed key
tiles, and head pairs (m, m+4) are emitted interleaved so their K=64
score matmuls can co-run in opposite halves of the PE array.
"""

import sys

sys.path.insert(0, "/opt/trn_rl_repo")

import numpy as np
import ml_dtypes

import concourse.bass as bass
import concourse.mybir as mybir
from concourse import bacc
from concourse.tile import TileContext
from concourse.bass_utils import run_bass_kernel_spmd
from concourse.masks import make_causal_mask

F32 = mybir.dt.float32
F32R = mybir.dt.float32r
BF16 = mybir.dt.bfloat16
U32 = mybir.dt.uint32
AF = mybir.ActivationFunctionType
ALU = mybir.AluOpType

# Problem constants
B, T, C = 4, 2048, 1024
WINDOW = 1024
EPS = 1.1920928955078125e-07
MAGIC = 0x5F3759DF

# Per-core constants (head-split SPMD)
NH = 8          # own q-heads
NKV = 2         # own kv-heads
D = 64
QDIM = NH * D   # 512
KVDIM = NKV * D  # 128
NT = T // 128   # 16 token tiles
NG = T // 256   # 8 query groups of 256
VSLOT = 128     # [v(64) | one | zeros] per kv head (128 for FWL)

_CACHE = {}


def _union_chunks(g):
    lo = max(0, 2 * g - 8)
    hi = min(NT - 1, 2 * g + 1)
    return list(range(lo, hi + 1))


def _qsel(c, g):
    """Column range [c0, c1) within group g covered by key chunk c."""
    lo_t = max(2 * g, c)
    hi_t = min(2 * g + 1, c + 8)
    if lo_t > hi_t:
        return None
    return ((lo_t - 2 * g) * 128, (hi_t + 1 - 2 * g) * 128)


def _segments(g):
    """Split union chunks into segments: runs of <=4 chunks with full
    (0,256) qsel, plus solo boundary chunks."""
    segs = []
    run = []
    for c in _union_chunks(g):
        if _qsel(c, g) == (0, 256):
            run.append(c)
            if len(run) == 4:
                segs.append(run)
                run = []
        else:
            if run:
                segs.append(run)
                run = []
            segs.append([c])
    if run:
        segs.append(run)
    return segs


def build_program(fake_collective=False):
    nc = bacc.Bacc("TRN2", target_bir_lowering=False, debug=False,
                   enable_asserts=True, num_devices=8)

    x_d = nc.dram_tensor("x_bf", [T, C], BF16, kind="ExternalInput")
    ve_d = nc.dram_tensor("ve_bf", [T, KVDIM], BF16, kind="ExternalInput")
    wq_d = nc.dram_tensor("wq_bf", [C, QDIM], BF16, kind="ExternalInput")
    wk_d = nc.dram_tensor("wk_bf", [C, KVDIM], BF16, kind="ExternalInput")
    wv_d = nc.dram_tensor("wv_bf", [C, KVDIM], BF16, kind="ExternalInput")
    wg_d = nc.dram_tensor("wg_bf", [8, NKV], BF16, kind="ExternalInput")
    wp_d = nc.dram_tensor("wp_bf", [C, 512], BF16, kind="ExternalInput")
    c2_d = nc.dram_tensor("c2_bf", [T, D], BF16, kind="ExternalInput")   # [cos|cos]
    s2_d = nc.dram_tensor("s2_bf", [T, D], BF16, kind="ExternalInput")   # [sin|-sin]
    ones_d = nc.dram_tensor("ones_f", [1, 64], F32, kind="ExternalInput")
    y_d = nc.dram_tensor("y_out", [T, 512], F32, kind="ExternalOutput")

    with TileContext(nc) as tc:
        with (
            tc.tile_pool(name="const", bufs=1) as constp,
            tc.tile_pool(name="persist", bufs=1) as pers,
        ):
            # ---- constants ----
            ident = constp.tile([128, 128], BF16, name="ident")
            nc.gpsimd.memset(ident[:, :], 0.0)
            nc.gpsimd.affine_select(
                out=ident[:, :], in_=ident[:, :],
                compare_op=ALU.not_equal, fill=1.0, base=0,
                pattern=[[-1, 128]], channel_multiplier=1)
            # diag mask lhsT: psum[tk,tq] += M[tq,tk]; -1e9 where tk > tq
            mdiag = constp.tile([128, 128], BF16, name="mdiag")
            make_causal_mask(nc, mdiag[:, :], mask_val=-1.0e9)
            # far mask: -1e9 where tk <= tq
            mfar = constp.tile([128, 128], BF16, name="mfar")
            nc.gpsimd.memset(mfar[:, :], 0.0)
            nc.gpsimd.affine_select(
                out=mfar[:, :], in_=mfar[:, :],
                compare_op=ALU.is_ge, fill=-1.0e9, base=-1,
                pattern=[[1, 128]], channel_multiplier=-1)
            magic = constp.tile([128, 1], U32, name="magic")
            nc.gpsimd.memset(magic[:, :], MAGIC)
            ones_r = constp.tile([1, 64], F32R, name="ones_r")
            nc.sync.dma_start(ones_r[:, :], ones_d.ap().bitcast(F32R))

            # ---- weights / rotary tables (one DMA each) ----
            wq_sb = pers.tile([128, 8, QDIM], BF16, name="wq_sb")
            wk_sb = pers.tile([128, 8, KVDIM], BF16, name="wk_sb")
            wv_sb = pers.tile([128, 8, KVDIM], BF16, name="wv_sb")
            wp_sb = pers.tile([128, 8, 512], BF16, name="wp_sb")
            wg_sb = pers.tile([8, NKV], BF16, name="wg_sb")
            c2_sb = pers.tile([128, NT, D], BF16, name="c2_sb")
            s2_sb = pers.tile([128, NT, D], BF16, name="s2_sb")
            ve_all = pers.tile([128, NT, KVDIM], BF16, name="ve_all")
            nc.sync.dma_start(wq_sb[:, :, :],
                              wq_d.ap().rearrange("(c p) n -> p c n", p=128))
            nc.sync.dma_start(wk_sb[:, :, :],
                              wk_d.ap().rearrange("(c p) n -> p c n", p=128))
            nc.sync.dma_start(wv_sb[:, :, :],
                              wv_d.ap().rearrange("(c p) n -> p c n", p=128))
            nc.sync.dma_start(wp_sb[:, :, :],
                              wp_d.ap().rearrange("(c p) n -> p c n", p=128))
            nc.sync.dma_start(wg_sb[:, :], wg_d.ap())
            nc.sync.dma_start(c2_sb[:, :, :],
                              c2_d.ap().rearrange("(t p) d -> p t d", p=128))
            nc.sync.dma_start(s2_sb[:, :, :],
                              s2_d.ap().rearrange("(t p) d -> p t d", p=128))
            nc.sync.dma_start(ve_all[:, :, :],
                              ve_d.ap().rearrange("(t p) d -> p t d", p=128))

            # persistent attention operands
            # qT[m] holds heads (m, m+4): partition half == kv half (j//4)
            qT = [pers.tile([128, T], BF16, name=f"qT{m}") for m in range(4)]
            kT = [pers.tile([128, 128], BF16, name=f"kTn{c}") for c in range(NT)]
            v_sb = [pers.tile([128, NKV * VSLOT], BF16, name=f"v{c}")
                    for c in range(NT)]

            # ============ phase 1: qkv / rope / norms / gating ===========
            with (
                tc.tile_pool(name="xtp", bufs=1) as xtp,
                tc.tile_pool(name="work", bufs=3) as work,
                tc.tile_pool(name="qps", bufs=2, space="PSUM") as qpsp,
                tc.tile_pool(name="kps", bufs=2, space="PSUM") as kpsp,
                tc.tile_pool(name="vps", bufs=2, space="PSUM") as vpsp,
                tc.tile_pool(name="zps", bufs=2, space="PSUM") as zpsp,
            ):
                # xT via DMA transpose (512-token pieces for pipelining)
                xT = [xtp.tile([128, 4, 512], BF16, name=f"xT{cc}")
                      for cc in range(8)]
                for tg in range(4):
                    for cc in range(8):
                        nc.sync.dma_start_transpose(
                            out=xT[cc][:, tg],
                            in_=x_d.ap()[tg * 512:(tg + 1) * 512,
                                         cc * 128:(cc + 1) * 128])
                for t in range(NT):
                    tg, to = t // 4, (t % 4) * 128
                    q_ps = qpsp.tile([128, QDIM], F32, tag="q_ps")
                    k_ps = kpsp.tile([128, KVDIM], F32, tag="k_ps")
                    v_ps = vpsp.tile([128, KVDIM], F32, tag="v_ps")
                    z_ps = zpsp.tile([128, NKV], F32, tag="z_ps")
                    for cc in range(8):
                        lhs = xT[cc][:, tg, to:to + 128]
                        st, sp_ = (cc == 0), (cc == 7)
                        nc.tensor.matmul(q_ps[:, :], lhs, wq_sb[:, cc],
                                         start=st, stop=sp_)
                        nc.tensor.matmul(k_ps[:, :], lhs, wk_sb[:, cc],
                                         start=st, stop=sp_)
                        nc.tensor.matmul(v_ps[:, :], lhs, wv_sb[:, cc],
                                         start=st, stop=sp_)
                    nc.tensor.matmul(z_ps[:, :],
                                     xT[0][0:8, tg, to:to + 128],
                                     wg_sb[:, :], start=True, stop=True)

                    # sums of squares on ScalarE (Square + accum from PSUM)
                    ms = work.tile([128, NH + NKV], F32, tag="ms")
                    sqd = work.tile([128, D], BF16, tag="sqd")
                    for j in range(NH):
                        nc.scalar.activation(
                            sqd[:, :], q_ps[:, j * D:(j + 1) * D], AF.Square,
                            accum_out=ms[:, j:j + 1])
                    for u in range(NKV):
                        nc.scalar.activation(
                            sqd[:, :], k_ps[:, u * D:(u + 1) * D], AF.Square,
                            accum_out=ms[:, NH + u:NH + u + 1])

                    # rs = rsqrt(ms + 64eps): bit-trick + 2 Newton iterations
                    NR = NH + NKV
                    a = work.tile([128, NR], F32, tag="a")
                    nc.vector.tensor_scalar_add(a[:, :], ms[:, :], 64.0 * EPS)
                    s1 = work.tile([128, NR], U32, tag="s1")
                    nc.vector.tensor_single_scalar(
                        s1[:, :], a[:, :].bitcast(U32), 1,
                        ALU.logical_shift_right)
                    r0 = work.tile([128, NR], F32, tag="r0")
                    nc.vector.tensor_tensor(
                        r0[:, :].bitcast(U32),
                        magic[:, :].to_broadcast((128, NR)).bitcast(U32),
                        s1[:, :], ALU.subtract)
                    t1 = work.tile([128, NR], F32, tag="t1")
                    rs = work.tile([128, NR], F32, tag="rs")
                    for it in range(2):
                        src = r0 if it == 0 else rs
                        nc.vector.tensor_mul(t1[:, :], src[:, :], src[:, :])
                        nc.vector.tensor_mul(t1[:, :], t1[:, :], a[:, :])
                        nc.vector.tensor_scalar(t1[:, :], t1[:, :],
                                                -0.5, 1.5, ALU.mult, ALU.add)
                        nc.vector.tensor_mul(rs[:, :], src[:, :], t1[:, :])

                    # q rope (DVE)
                    q_sb = work.tile([128, QDIM], BF16, tag="q_sb")
                    nc.vector.tensor_copy(q_sb[:, :], q_ps[:, :])
                    qg = q_sb[:, :].rearrange("p (h s f) -> p h s f", h=NH, s=2)
                    c2b = c2_sb[:, t].unsqueeze(1).broadcast_to([128, NH, D])
                    s2b = s2_sb[:, t].rearrange("p (s f) -> p s f", s=2)
                    m1 = work.tile([128, QDIM], BF16, tag="m1")
                    m1g = m1[:, :].rearrange("p (h s f) -> p h s f", h=NH, s=2)
                    nc.vector.tensor_mul(
                        m1g[:, :, 0], qg[:, :, 1],
                        s2b[:, 0].unsqueeze(1).broadcast_to([128, NH, 32]))
                    nc.vector.tensor_mul(
                        m1g[:, :, 1], qg[:, :, 0],
                        s2b[:, 1].unsqueeze(1).broadcast_to([128, NH, 32]))
                    rq = work.tile([128, QDIM], BF16, tag="rq")
                    nc.vector.tensor_mul(
                        rq[:, :].rearrange("p (h d) -> p h d", h=NH),
                        q_sb[:, :].rearrange("p (h d) -> p h d", h=NH), c2b)
                    nc.vector.tensor_add(rq[:, :], rq[:, :], m1[:, :])

                    # k rope (gpsimd)
                    k_sb = work.tile([128, KVDIM], BF16, tag="k_sb")
                    nc.vector.tensor_copy(k_sb[:, :], k_ps[:, :])
                    kg = k_sb[:, :].rearrange("p (u s f) -> p u s f", u=NKV, s=2)
                    km = work.tile([128, KVDIM], BF16, tag="km")
                    kmg = km[:, :].rearrange("p (u s f) -> p u s f", u=NKV, s=2)
                    nc.gpsimd.tensor_mul(
                        kmg[:, :, 0], kg[:, :, 1],
                        s2b[:, 0].unsqueeze(1).broadcast_to([128, NKV, 32]))
                    nc.gpsimd.tensor_mul(
                        kmg[:, :, 1], kg[:, :, 0],
                        s2b[:, 1].unsqueeze(1).broadcast_to([128, NKV, 32]))
                    rk = work.tile([128, KVDIM], BF16, tag="rk")
                    nc.gpsimd.tensor_mul(
                        rk[:, :].rearrange("p (u d) -> p u d", u=NKV),
                        k_sb[:, :].rearrange("p (u d) -> p u d", u=NKV),
                        c2_sb[:, t].unsqueeze(1).broadcast_to([128, NKV, D]))
                    nc.gpsimd.tensor_add(rk[:, :], rk[:, :], km[:, :])

                    # gate = 2*sigmoid(z) = 2/(1+exp(-z))
                    e_sb = work.tile([128, NKV], F32, tag="e_sb")
                    nc.scalar.activation(e_sb[:, :], z_ps[:, :], AF.Exp,
                                         bias=0.0, scale=-1.0)
                    gp1 = work.tile([128, NKV], F32, tag="gp1")
                    nc.vector.tensor_scalar_add(gp1[:, :], e_sb[:, :], 1.0)
                    gr = work.tile([128, NKV], F32, tag="gr")
                    nc.vector.reciprocal(gr[:, :], gp1[:, :])

                    # v' = v + 2*gate*ve  into [v|one|zeros] slots
                    ves = work.tile([128, KVDIM], BF16, tag="ves")
                    for u in range(NKV):
                        nc.vector.tensor_scalar(
                            ves[:, u * D:(u + 1) * D],
                            ve_all[:, t, u * D:(u + 1) * D],
                            gr[:, u:u + 1], 2.0, ALU.mult, ALU.mult)
                    vv = v_sb[t][:, :].rearrange("p (u s) -> p u s", u=NKV)
                    nc.vector.tensor_tensor(
                        vv[:, :, 0:D],
                        v_ps[:, :].rearrange("p (u d) -> p u d", u=NKV),
                        ves[:, :].rearrange("p (u d) -> p u d", u=NKV), ALU.add)
                    nc.gpsimd.memset(vv[:, :, D:D + 1], 1.0)
                    nc.gpsimd.memset(vv[:, :, D + 1:VSLOT], 0.0)

                    # k~ = rope(k) * (8*rs_k), transposed
                    krs = work.tile([128, KVDIM], BF16, tag="krs")
                    for u in range(NKV):
                        nc.vector.tensor_scalar(
                            krs[:, u * D:(u + 1) * D],
                            rk[:, u * D:(u + 1) * D],
                            rs[:, NH + u:NH + u + 1], 8.0, ALU.mult, ALU.mult)
                    nc.sync.dma_start_transpose(out=kT[t][:, :], in_=krs[:, :])

                    # q~ = rope(q) * rs_q (gpsimd), slotted so head j lands at
                    # column (j%4)*128 + (j//4)*64, then transposed per pair
                    qn = work.tile([128, QDIM], BF16, tag="qn")
                    for j in range(NH):
                        o = (j % 4) * 128 + (j // 4) * 64
                        nc.gpsimd.tensor_scalar_mul(
                            qn[:, o:o + D], rq[:, j * D:(j + 1) * D],
                            rs[:, j:j + 1])
                    for m in range(4):
                        nc.sync.dma_start_transpose(
                            out=qT[m][:, t * 128:(t + 1) * 128],
                            in_=qn[:, m * 128:(m + 1) * 128])

            # ======== phase 2: attention + pipelined exchange + proj ======
            with (
                tc.tile_pool(name="probs", bufs=10) as probp,
                tc.tile_pool(name="yug", bufs=2) as yugp,
                tc.tile_pool(name="att2", bufs=4) as att2,
                tc.tile_pool(name="ytg", bufs=3) as ytgp,
                tc.tile_pool(name="proj_sb", bufs=3) as projp,
                tc.tile_pool(name="oall", bufs=1) as oallp,
                tc.tile_pool(name="scps", bufs=2, space="PSUM") as scpsp,
                tc.tile_pool(name="ytps", bufs=2, space="PSUM") as ytpsp,
                tc.tile_pool(name="rbps", bufs=1, space="PSUM") as rbpsp,
                tc.tile_pool(name="prps", bufs=1, space="PSUM") as prpsp,
                tc.tile_pool(name="dram2", bufs=3, space="DRAM") as dram2p,
            ):
                o_all = oallp.tile([128, NT, 512], F32, name="o_all")
                for g in range(NG):
                    segs = _segments(g)
                    cs = _union_chunks(g)
                    yu_g = yugp.tile([VSLOT, NH * 256], F32, tag="yu_g")
                    for m in range(4):
                        jpair = (m, m + 4)
                        pieces = {j: [] for j in jpair}
                        for seg in segs:
                            scs = {}
                            pts = {}
                            for j in jpair:
                                scs[j] = scpsp.tile([128, 1024], F32, tag="sc", name=f"sc{j}")
                                pts[j] = probp.tile([128, 1024], BF16,
                                                    tag="pt", name=f"pt{j}")
                            for pos, c in enumerate(seg):
                                c0, c1 = _qsel(c, g)
                                o = pos * 256
                                # the two heads' K=64 matmuls use opposite
                                # partition halves -> co-run on the PE array
                                for j in jpair:
                                    base = (j // 4) * 64
                                    nc.tensor.matmul(
                                        scs[j][:, o + c0:o + c1],
                                        kT[c][base:base + 64, :],
                                        qT[m][base:base + 64,
                                              g * 256 + c0:g * 256 + c1],
                                        start=True, stop=True)
                                for j in jpair:
                                    if c in (2 * g, 2 * g + 1):      # diag
                                        d0 = o + (c - 2 * g) * 128
                                        nc.tensor.matmul(
                                            scs[j][:, d0:d0 + 128],
                                            mdiag[:, :], ident[:, :],
                                            start=False, stop=True,
                                            skip_group_check=True)
                                    if c + 8 in (2 * g, 2 * g + 1):  # far
                                        f0 = o + (c + 8 - 2 * g) * 128
                                        nc.tensor.matmul(
                                            scs[j][:, f0:f0 + 128],
                                            mfar[:, :], ident[:, :],
                                            start=False, stop=True,
                                            skip_group_check=True)
                                    if (c0, c1) != (0, 256):
                                        if c0 > 0:
                                            nc.gpsimd.memset(
                                                pts[j][:, o:o + c0], 0.0)
                                        if c1 < 256:
                                            nc.gpsimd.memset(
                                                pts[j][:, o + c1:o + 256], 0.0)
                                    pieces[j].append((pts[j], o))
                            lo = min(_qsel(c, g)[0] + 256 * p2
                                     for p2, c in enumerate(seg))
                            hi = max(_qsel(c, g)[1] + 256 * p2
                                     for p2, c in enumerate(seg))
                            for j in jpair:
                                nc.scalar.activation(
                                    pts[j][:, lo:hi], scs[j][:, lo:hi],
                                    AF.Exp, bias=0.0, scale=1.0)
                        for j in jpair:
                            u = j // 4
                            yt = ytpsp.tile([VSLOT, 256], F32, tag="yt")
                            for ci, c in enumerate(cs):
                                pt, o = pieces[j][ci]
                                nc.tensor.matmul(
                                    yt[:, :],
                                    v_sb[c][:, u * VSLOT:(u + 1) * VSLOT],
                                    pt[:, o:o + 256],
                                    start=(ci == 0), stop=(ci == len(cs) - 1))
                            nc.vector.tensor_copy(
                                yu_g[:, j * 256:(j + 1) * 256], yt[0:VSLOT, :])
                    # softmax denominators: gather, reciprocal, broadcast
                    sump = att2.tile([NH, 256], F32, tag="sump")
                    nc.sync.dma_start(sump[:, :], yu_g[64:65, :])
                    rec = att2.tile([NH, 256], F32R, tag="rec")
                    with nc.allow_low_precision(reason="fp32r recip broadcast"):
                        nc.vector.reciprocal(rec[:, :], sump[:, :])
                    recf = att2.tile([1, NH * 256], F32R, tag="recf")
                    nc.sync.dma_start(recf[:, :], rec[:, :])
                    ytg = [ytgp.tile([128, 256], BF16, tag=f"ytg{m}",
                                     name=f"ytg{g}_{m}")
                           for m in range(4)]
                    for j in range(NH):
                        rb = rbpsp.tile([64, 256], F32, tag="rb")
                        nc.tensor.matmul(rb[:, :], ones_r[:, :],
                                         recf[:, j * 256:(j + 1) * 256],
                                         start=True, stop=True)
                        m = j // 2
                        if j % 2 == 0:
                            nc.vector.tensor_mul(
                                ytg[m][0:64, :],
                                yu_g[0:64, j * 256:(j + 1) * 256], rb[:, :])
                        else:
                            yn = att2.tile([64, 256], BF16, tag="yn")
                            nc.vector.tensor_mul(
                                yn[:, :], yu_g[0:64, j * 256:(j + 1) * 256],
                                rb[:, :])
                            nc.sync.dma_start(ytg[m][64:128, :], yn[:, :])
                    # pipelined exchange for this query group
                    agin = dram2p.tile([4, 128, 256], BF16, tag="agin")
                    agout = dram2p.tile([8, 128, 256], BF16, tag="agout")
                    for m in range(4):
                        nc.sync.dma_start(agin[m], ytg[m][:, :])
                    if fake_collective:
                        nc.sync.dma_start(agout[0:4], agin[:, :, :])
                        nc.sync.dma_start(agout[4:8], agin[:, :, :])
                    else:
                        nc.gpsimd.collective_compute(
                            "AllGather", ALU.bypass,
                            replica_groups=[[0, 1], [2, 3], [4, 5], [6, 7]],
                            ins=[agin[:, :, :].opt()],
                            outs=[agout[:, :, :].opt()])
                    ygs = projp.tile([128, 8, 256], BF16, tag="ygs")
                    nc.sync.dma_start(
                        ygs[:, :, :],
                        agout[:, :, :].rearrange("c p n -> p c n"))
                    for i in (2 * g, 2 * g + 1):
                        io = (i - 2 * g) * 128
                        pr = prpsp.tile([128, 512], F32, tag="pr")
                        for cc in range(8):
                            nc.tensor.matmul(pr[:, :],
                                             ygs[:, cc, io:io + 128],
                                             wp_sb[:, cc],
                                             start=(cc == 0), stop=(cc == 7))
                        nc.vector.tensor_copy(o_all[:, i, :], pr[:, :])
                    nc.sync.dma_start(
                        y_d.ap()[g * 256:(g + 1) * 256, :].rearrange(
                            "(i p) n -> p i n", p=128),
                        o_all[:, 2 * g:2 * g + 2, :])

    nc.compile()
    return nc


def _prep_inputs(x, ve, cos, sin, wq, wk, wv, wproj, wgate):
    bf = ml_dtypes.bfloat16
    cosf = np.asarray(cos, np.float32).reshape(T, 32)
    sinf = np.asarray(sin, np.float32).reshape(T, 32)
    c2 = np.concatenate([cosf, cosf], axis=1).astype(bf)
    s2 = np.concatenate([sinf, -sinf], axis=1).astype(bf)
    x = np.asarray(x, np.float32)
    ve = np.asarray(ve, np.float32)
    wq = np.asarray(wq, np.float32)
    wk = np.asarray(wk, np.float32)
    wv = np.asarray(wv, np.float32)
    wproj = np.asarray(wproj, np.float32)
    wgate = np.asarray(wgate, np.float32)
    maps = []
    for core in range(8):
        b, hp = core // 2, core % 2
        maps.append({
            "x_bf": x[b].astype(bf),
            "ve_bf": ve[b][:, hp * 128:(hp + 1) * 128].astype(bf),
            "wq_bf": wq[:, hp * 512:(hp + 1) * 512].astype(bf),
            "wk_bf": wk[:, hp * 128:(hp + 1) * 128].astype(bf),
            "wv_bf": wv[:, hp * 128:(hp + 1) * 128].astype(bf),
            "wg_bf": wgate[:, hp * 2:(hp + 1) * 2].astype(bf),
            "wp_bf": wproj[:, hp * 512:(hp + 1) * 512].astype(bf),
            "c2_bf": c2,
            "s2_bf": s2,
            "ones_f": np.ones((1, 64), np.float32),
        })
    return maps


def kernel(x, ve, cos, sin, wq, wk, wv, wproj, wgate, window):
    assert int(window) == WINDOW
    if "nc" not in _CACHE:
        _CACHE["nc"] = build_program()
    nc = _CACHE["nc"]
    maps = _prep_inputs(x, ve, cos, sin, wq, wk, wv, wproj, wgate)
    res = run_bass_kernel_spmd(nc, maps, list(range(8))).results
    y = np.empty((B, T, C), np.float32)
    for core in range(8):
        b, hp = core // 2, core % 2
        y[b][:, hp * 512:(hp + 1) * 512] = res[core]["y_out"]
    return y


# revision 16
# speedup vs baseline: 17.4629x; 12.7434x over previous
"""Trainium2 Bass kernel for nn_CausalSelfAttention_12283606468211.

Sliding-window causal GQA attention (B=4, T=2048, C=1024, 16 q-heads,
4 kv-heads, head_dim 64, window 1024) with value-embedding gating,
RoPE + QK-RMSNorm, and output projection.

Sharding: 8 cores = 4 batches x 2 head-halves.  Core (2b+p) computes
q-heads [8p, 8p+8) and kv-heads [2p, 2p+2) of batch b for ALL 2048
queries.  The per-head attention outputs are exchanged between the two
cores of a batch with small pipelined AllGathers (one per 256-query
group) and each core computes its half of the output columns of the
final projection.  The SPMD program is identical on every core; all
per-core differences are carried by host-side input slicing.

All matmuls run in bf16 (inputs pre-cast on host); accumulation is
fp32 in PSUM.  Softmax needs no max-subtraction because QK-RMSNorm
bounds |score| <= 8.  Sum-of-squares for the RMSNorms run on ScalarE
(Square + accum_out, straight from PSUM); the reciprocal square roots
are computed on the vector engine (bit-trick seed + 2 Newton
iterations), so the only ScalarE table set ever loaded is Exp's.  The
RMSNorm scales fold into q~ and k~ before the QK^T matmuls, so Exp
runs with a constant scale and scores for up to 4 key chunks share one
wide Exp op.  The softmax denominator comes free from an appended
ones-column in the PV matmul; its reciprocal is broadcast across
partitions with a K=1 fp32r matmul.  Causal/window masking accumulates
-1e9 triangle matrices into the score PSUM via constant matmuls.
Query heads are slotted as (j%4, j//4) in the transposed tiles so each
head's partition half matches its kv-head's half in the transposed key
tiles, and head pairs (m, m+4) are emitted interleaved so their K=64
score matmuls can co-run in opposite halves of the PE array.
"""

import sys

sys.path.insert(0, "/opt/trn_rl_repo")

import numpy as np
import ml_dtypes

import concourse.bass as bass
import concourse.mybir as mybir
from concourse import bacc
from concourse.tile import TileContext
from concourse.bass_utils import run_bass_kernel_spmd
from concourse.masks import make_causal_mask

F32 = mybir.dt.float32
F32R = mybir.dt.float32r
BF16 = mybir.dt.bfloat16
U32 = mybir.dt.uint32
AF = mybir.ActivationFunctionType
ALU = mybir.AluOpType

# Problem constants
B, T, C = 4, 2048, 1024
WINDOW = 1024
EPS = 1.1920928955078125e-07
MAGIC = 0x5F3759DF

# Per-core constants (head-split SPMD)
NH = 8          # own q-heads
NKV = 2         # own kv-heads
D = 64
QDIM = NH * D   # 512
KVDIM = NKV * D  # 128
NT = T // 128   # 16 token tiles
NG = T // 256   # 8 query groups of 256
VSLOT = 128     # [v(64) | one | zeros] per kv head (128 for FWL)

_CACHE = {}


def _union_chunks(g):
    lo = max(0, 2 * g - 8)
    hi = min(NT - 1, 2 * g + 1)
    return list(range(lo, hi + 1))


def _qsel(c, g):
    """Column range [c0, c1) within group g covered by key chunk c."""
    lo_t = max(2 * g, c)
    hi_t = min(2 * g + 1, c + 8)
    if lo_t > hi_t:
        return None
    return ((lo_t - 2 * g) * 128, (hi_t + 1 - 2 * g) * 128)


def _segments(g):
    """Split union chunks into segments: runs of <=4 chunks with full
    (0,256) qsel, plus solo boundary chunks."""
    segs = []
    run = []
    for c in _union_chunks(g):
        if _qsel(c, g) == (0, 256):
            run.append(c)
            if len(run) == 4:
                segs.append(run)
                run = []
        else:
            if run:
                segs.append(run)
                run = []
            segs.append([c])
    if run:
        segs.append(run)
    return segs


def build_floor_program():
    """Same I/O signature, trivial body — for matched-payload timing."""
    nc = bacc.Bacc("TRN2", target_bir_lowering=False, debug=False,
                   enable_asserts=True, num_devices=8)
    x_d = nc.dram_tensor("x_bf", [T, C], BF16, kind="ExternalInput")
    ve_d = nc.dram_tensor("ve_bf", [T, KVDIM], BF16, kind="ExternalInput")
    wq_d = nc.dram_tensor("wq_bf", [C, QDIM], BF16, kind="ExternalInput")
    wk_d = nc.dram_tensor("wk_bf", [C, KVDIM], BF16, kind="ExternalInput")
    wv_d = nc.dram_tensor("wv_bf", [C, KVDIM], BF16, kind="ExternalInput")
    wg_d = nc.dram_tensor("wg_bf", [8, NKV], BF16, kind="ExternalInput")
    wp_d = nc.dram_tensor("wp_bf", [C, 512], BF16, kind="ExternalInput")
    c2_d = nc.dram_tensor("c2_bf", [T, D], BF16, kind="ExternalInput")
    s2_d = nc.dram_tensor("s2_bf", [T, D], BF16, kind="ExternalInput")
    ones_d = nc.dram_tensor("ones_f", [1, 64], F32, kind="ExternalInput")
    y_d = nc.dram_tensor("y_out", [T, 512], F32, kind="ExternalOutput")
    with TileContext(nc) as tc:
        with tc.tile_pool(name="p", bufs=2) as pool:
            t = pool.tile([128, 256], BF16, name="t")
            nc.sync.dma_start(t[:, :], x_d.ap()[0:128, 0:256])
            t2 = pool.tile([128, 256], F32, name="t2")
            nc.vector.tensor_copy(t2[:, :], t[:, :])
            nc.sync.dma_start(y_d.ap()[0:128, 0:256], t2[:, :])
    nc.compile()
    return nc


def build_program(fake_collective=False):
    nc = bacc.Bacc("TRN2", target_bir_lowering=False, debug=False,
                   enable_asserts=True, num_devices=8)

    x_d = nc.dram_tensor("x_bf", [T, C], BF16, kind="ExternalInput")
    ve_d = nc.dram_tensor("ve_bf", [T, KVDIM], BF16, kind="ExternalInput")
    wq_d = nc.dram_tensor("wq_bf", [C, QDIM], BF16, kind="ExternalInput")
    wk_d = nc.dram_tensor("wk_bf", [C, KVDIM], BF16, kind="ExternalInput")
    wv_d = nc.dram_tensor("wv_bf", [C, KVDIM], BF16, kind="ExternalInput")
    wg_d = nc.dram_tensor("wg_bf", [8, NKV], BF16, kind="ExternalInput")
    wp_d = nc.dram_tensor("wp_bf", [C, 512], BF16, kind="ExternalInput")
    c2_d = nc.dram_tensor("c2_bf", [T, D], BF16, kind="ExternalInput")   # [cos|cos]
    s2_d = nc.dram_tensor("s2_bf", [T, D], BF16, kind="ExternalInput")   # [sin|-sin]
    ones_d = nc.dram_tensor("ones_f", [1, 64], F32, kind="ExternalInput")
    y_d = nc.dram_tensor("y_out", [T, 512], F32, kind="ExternalOutput")

    with TileContext(nc) as tc:
        with (
            tc.tile_pool(name="const", bufs=1) as constp,
            tc.tile_pool(name="persist", bufs=1) as pers,
        ):
            # ---- constants ----
            ident = constp.tile([128, 128], BF16, name="ident")
            nc.gpsimd.memset(ident[:, :], 0.0)
            nc.gpsimd.affine_select(
                out=ident[:, :], in_=ident[:, :],
                compare_op=ALU.not_equal, fill=1.0, base=0,
                pattern=[[-1, 128]], channel_multiplier=1)
            # diag mask lhsT: psum[tk,tq] += M[tq,tk]; -1e9 where tk > tq
            mdiag = constp.tile([128, 128], BF16, name="mdiag")
            make_causal_mask(nc, mdiag[:, :], mask_val=-1.0e9)
            # far mask: -1e9 where tk <= tq
            mfar = constp.tile([128, 128], BF16, name="mfar")
            nc.gpsimd.memset(mfar[:, :], 0.0)
            nc.gpsimd.affine_select(
                out=mfar[:, :], in_=mfar[:, :],
                compare_op=ALU.is_ge, fill=-1.0e9, base=-1,
                pattern=[[1, 128]], channel_multiplier=-1)
            magic = constp.tile([128, 1], U32, name="magic")
            nc.gpsimd.memset(magic[:, :], MAGIC)
            ones_r = constp.tile([1, 64], F32R, name="ones_r")
            nc.sync.dma_start(ones_r[:, :], ones_d.ap().bitcast(F32R))

            # ---- weights / rotary tables (one DMA each) ----
            wq_sb = pers.tile([128, 8, QDIM], BF16, name="wq_sb")
            wk_sb = pers.tile([128, 8, KVDIM], BF16, name="wk_sb")
            wv_sb = pers.tile([128, 8, KVDIM], BF16, name="wv_sb")
            wp_sb = pers.tile([128, 8, 512], BF16, name="wp_sb")
            wg_sb = pers.tile([8, NKV], BF16, name="wg_sb")
            c2_sb = pers.tile([128, NT, D], BF16, name="c2_sb")
            s2_sb = pers.tile([128, NT, D], BF16, name="s2_sb")
            ve_all = pers.tile([128, NT, KVDIM], BF16, name="ve_all")
            nc.sync.dma_start(wq_sb[:, :, :],
                              wq_d.ap().rearrange("(c p) n -> p c n", p=128))
            nc.sync.dma_start(wk_sb[:, :, :],
                              wk_d.ap().rearrange("(c p) n -> p c n", p=128))
            nc.sync.dma_start(wv_sb[:, :, :],
                              wv_d.ap().rearrange("(c p) n -> p c n", p=128))
            nc.sync.dma_start(wp_sb[:, :, :],
                              wp_d.ap().rearrange("(c p) n -> p c n", p=128))
            nc.sync.dma_start(wg_sb[:, :], wg_d.ap())
            nc.sync.dma_start(c2_sb[:, :, :],
                              c2_d.ap().rearrange("(t p) d -> p t d", p=128))
            nc.sync.dma_start(s2_sb[:, :, :],
                              s2_d.ap().rearrange("(t p) d -> p t d", p=128))
            nc.sync.dma_start(ve_all[:, :, :],
                              ve_d.ap().rearrange("(t p) d -> p t d", p=128))

            # persistent attention operands
            # qT[m] holds heads (m, m+4): partition half == kv half (j//4)
            qT = [pers.tile([128, T], BF16, name=f"qT{m}") for m in range(4)]
            kT = [pers.tile([128, 128], BF16, name=f"kTn{c}") for c in range(NT)]
            v_sb = [pers.tile([128, NKV * VSLOT], BF16, name=f"v{c}")
                    for c in range(NT)]

            # ============ phase 1: qkv / rope / norms / gating ===========
            with (
                tc.tile_pool(name="xtp", bufs=1) as xtp,
                tc.tile_pool(name="work", bufs=3) as work,
                tc.tile_pool(name="qps", bufs=2, space="PSUM") as qpsp,
                tc.tile_pool(name="kps", bufs=2, space="PSUM") as kpsp,
                tc.tile_pool(name="vps", bufs=2, space="PSUM") as vpsp,
                tc.tile_pool(name="zps", bufs=2, space="PSUM") as zpsp,
            ):
                # xT via DMA transpose (512-token pieces for pipelining)
                xT = [xtp.tile([128, 4, 512], BF16, name=f"xT{cc}")
                      for cc in range(8)]
                for tg in range(4):
                    for cc in range(8):
                        nc.sync.dma_start_transpose(
                            out=xT[cc][:, tg],
                            in_=x_d.ap()[tg * 512:(tg + 1) * 512,
                                         cc * 128:(cc + 1) * 128])
                for t in range(NT):
                    tg, to = t // 4, (t % 4) * 128
                    q_ps = qpsp.tile([128, QDIM], F32, tag="q_ps")
                    k_ps = kpsp.tile([128, KVDIM], F32, tag="k_ps")
                    v_ps = vpsp.tile([128, KVDIM], F32, tag="v_ps")
                    z_ps = zpsp.tile([128, NKV], F32, tag="z_ps")
                    for cc in range(8):
                        lhs = xT[cc][:, tg, to:to + 128]
                        st, sp_ = (cc == 0), (cc == 7)
                        nc.tensor.matmul(q_ps[:, :], lhs, wq_sb[:, cc],
                                         start=st, stop=sp_)
                        nc.tensor.matmul(k_ps[:, :], lhs, wk_sb[:, cc],
                                         start=st, stop=sp_)
                        nc.tensor.matmul(v_ps[:, :], lhs, wv_sb[:, cc],
                                         start=st, stop=sp_)
                    nc.tensor.matmul(z_ps[:, :],
                                     xT[0][0:8, tg, to:to + 128],
                                     wg_sb[:, :], start=True, stop=True)

                    # sums of squares on ScalarE (Square + accum from PSUM)
                    ms = work.tile([128, NH + NKV], F32, tag="ms")
                    sqd = work.tile([128, D], BF16, tag="sqd")
                    for j in range(NH):
                        nc.scalar.activation(
                            sqd[:, :], q_ps[:, j * D:(j + 1) * D], AF.Square,
                            accum_out=ms[:, j:j + 1])
                    for u in range(NKV):
                        nc.scalar.activation(
                            sqd[:, :], k_ps[:, u * D:(u + 1) * D], AF.Square,
                            accum_out=ms[:, NH + u:NH + u + 1])

                    # rs = rsqrt(ms + 64eps): bit-trick + 2 Newton iterations
                    NR = NH + NKV
                    a = work.tile([128, NR], F32, tag="a")
                    nc.vector.tensor_scalar_add(a[:, :], ms[:, :], 64.0 * EPS)
                    s1 = work.tile([128, NR], U32, tag="s1")
                    nc.vector.tensor_single_scalar(
                        s1[:, :], a[:, :].bitcast(U32), 1,
                        ALU.logical_shift_right)
                    r0 = work.tile([128, NR], F32, tag="r0")
                    nc.vector.tensor_tensor(
                        r0[:, :].bitcast(U32),
                        magic[:, :].to_broadcast((128, NR)).bitcast(U32),
                        s1[:, :], ALU.subtract)
                    t1 = work.tile([128, NR], F32, tag="t1")
                    rs = work.tile([128, NR], F32, tag="rs")
                    for it in range(2):
                        src = r0 if it == 0 else rs
                        nc.vector.tensor_mul(t1[:, :], src[:, :], src[:, :])
                        nc.vector.tensor_mul(t1[:, :], t1[:, :], a[:, :])
                        nc.vector.tensor_scalar(t1[:, :], t1[:, :],
                                                -0.5, 1.5, ALU.mult, ALU.add)
                        nc.vector.tensor_mul(rs[:, :], src[:, :], t1[:, :])

                    # q rope (DVE)
                    q_sb = work.tile([128, QDIM], BF16, tag="q_sb")
                    nc.vector.tensor_copy(q_sb[:, :], q_ps[:, :])
                    qg = q_sb[:, :].rearrange("p (h s f) -> p h s f", h=NH, s=2)
                    c2b = c2_sb[:, t].unsqueeze(1).broadcast_to([128, NH, D])
                    s2b = s2_sb[:, t].rearrange("p (s f) -> p s f", s=2)
                    m1 = work.tile([128, QDIM], BF16, tag="m1")
                    m1g = m1[:, :].rearrange("p (h s f) -> p h s f", h=NH, s=2)
                    nc.vector.tensor_mul(
                        m1g[:, :, 0], qg[:, :, 1],
                        s2b[:, 0].unsqueeze(1).broadcast_to([128, NH, 32]))
                    nc.vector.tensor_mul(
                        m1g[:, :, 1], qg[:, :, 0],
                        s2b[:, 1].unsqueeze(1).broadcast_to([128, NH, 32]))
                    rq = work.tile([128, QDIM], BF16, tag="rq")
                    nc.vector.tensor_mul(
                        rq[:, :].rearrange("p (h d) -> p h d", h=NH),
                        q_sb[:, :].rearrange("p (h d) -> p h d", h=NH), c2b)
                    nc.vector.tensor_add(rq[:, :], rq[:, :], m1[:, :])

                    # k rope (gpsimd)
                    k_sb = work.tile([128, KVDIM], BF16, tag="k_sb")
                    nc.vector.tensor_copy(k_sb[:, :], k_ps[:, :])
                    kg = k_sb[:, :].rearrange("p (u s f) -> p u s f", u=NKV, s=2)
                    km = work.tile([128, KVDIM], BF16, tag="km")
                    kmg = km[:, :].rearrange("p (u s f) -> p u s f", u=NKV, s=2)
                    nc.gpsimd.tensor_mul(
                        kmg[:, :, 0], kg[:, :, 1],
                        s2b[:, 0].unsqueeze(1).broadcast_to([128, NKV, 32]))
                    nc.gpsimd.tensor_mul(
                        kmg[:, :, 1], kg[:, :, 0],
                        s2b[:, 1].unsqueeze(1).broadcast_to([128, NKV, 32]))
                    rk = work.tile([128, KVDIM], BF16, tag="rk")
                    nc.gpsimd.tensor_mul(
                        rk[:, :].rearrange("p (u d) -> p u d", u=NKV),
                        k_sb[:, :].rearrange("p (u d) -> p u d", u=NKV),
                        c2_sb[:, t].unsqueeze(1).broadcast_to([128, NKV, D]))
                    nc.gpsimd.tensor_add(rk[:, :], rk[:, :], km[:, :])

                    # gate = 2*sigmoid(z) = 2/(1+exp(-z))
                    e_sb = work.tile([128, NKV], F32, tag="e_sb")
                    nc.scalar.activation(e_sb[:, :], z_ps[:, :], AF.Exp,
                                         bias=0.0, scale=-1.0)
                    gp1 = work.tile([128, NKV], F32, tag="gp1")
                    nc.vector.tensor_scalar_add(gp1[:, :], e_sb[:, :], 1.0)
                    gr = work.tile([128, NKV], F32, tag="gr")
                    nc.vector.reciprocal(gr[:, :], gp1[:, :])

                    # v' = v + 2*gate*ve  into [v|one|zeros] slots
                    ves = work.tile([128, KVDIM], BF16, tag="ves")
                    for u in range(NKV):
                        nc.vector.tensor_scalar(
                            ves[:, u * D:(u + 1) * D],
                            ve_all[:, t, u * D:(u + 1) * D],
                            gr[:, u:u + 1], 2.0, ALU.mult, ALU.mult)
                    vv = v_sb[t][:, :].rearrange("p (u s) -> p u s", u=NKV)
                    nc.vector.tensor_tensor(
                        vv[:, :, 0:D],
                        v_ps[:, :].rearrange("p (u d) -> p u d", u=NKV),
                        ves[:, :].rearrange("p (u d) -> p u d", u=NKV), ALU.add)
                    nc.gpsimd.memset(vv[:, :, D:D + 1], 1.0)
                    nc.gpsimd.memset(vv[:, :, D + 1:VSLOT], 0.0)

                    # k~ = rope(k) * (8*rs_k), transposed
                    krs = work.tile([128, KVDIM], BF16, tag="krs")
                    for u in range(NKV):
                        nc.vector.tensor_scalar(
                            krs[:, u * D:(u + 1) * D],
                            rk[:, u * D:(u + 1) * D],
                            rs[:, NH + u:NH + u + 1], 8.0, ALU.mult, ALU.mult)
                    nc.sync.dma_start_transpose(out=kT[t][:, :], in_=krs[:, :])

                    # q~ = rope(q) * rs_q (gpsimd), slotted so head j lands at
                    # column (j%4)*128 + (j//4)*64, then transposed per pair
                    qn = work.tile([128, QDIM], BF16, tag="qn")
                    for j in range(NH):
                        o = (j % 4) * 128 + (j // 4) * 64
                        nc.gpsimd.tensor_scalar_mul(
                            qn[:, o:o + D], rq[:, j * D:(j + 1) * D],
                            rs[:, j:j + 1])
                    for m in range(4):
                        nc.sync.dma_start_transpose(
                            out=qT[m][:, t * 128:(t + 1) * 128],
                            in_=qn[:, m * 128:(m + 1) * 128])

            # ======== phase 2: attention + pipelined exchange + proj ======
            with (
                tc.tile_pool(name="probs", bufs=10) as probp,
                tc.tile_pool(name="yug", bufs=2) as yugp,
                tc.tile_pool(name="att2", bufs=4) as att2,
                tc.tile_pool(name="ytg", bufs=3) as ytgp,
                tc.tile_pool(name="proj_sb", bufs=3) as projp,
                tc.tile_pool(name="oall", bufs=1) as oallp,
                tc.tile_pool(name="scps", bufs=2, space="PSUM") as scpsp,
                tc.tile_pool(name="ytps", bufs=2, space="PSUM") as ytpsp,
                tc.tile_pool(name="rbps", bufs=1, space="PSUM") as rbpsp,
                tc.tile_pool(name="prps", bufs=1, space="PSUM") as prpsp,
                tc.tile_pool(name="dram2", bufs=3, space="DRAM") as dram2p,
            ):
                o_all = oallp.tile([128, NT, 512], F32, name="o_all")
                for g in range(NG):
                    segs = _segments(g)
                    cs = _union_chunks(g)
                    yu_g = yugp.tile([VSLOT, NH * 256], F32, tag="yu_g")
                    for m in range(4):
                        jpair = (m, m + 4)
                        pieces = {j: [] for j in jpair}
                        for seg in segs:
                            scs = {}
                            pts = {}
                            for j in jpair:
                                scs[j] = scpsp.tile([128, 1024], F32, tag="sc", name=f"sc{j}")
                                pts[j] = probp.tile([128, 1024], BF16,
                                                    tag="pt", name=f"pt{j}")
                            for pos, c in enumerate(seg):
                                c0, c1 = _qsel(c, g)
                                o = pos * 256
                                # the two heads' K=64 matmuls use opposite
                                # partition halves -> co-run on the PE array
                                for j in jpair:
                                    base = (j // 4) * 64
                                    nc.tensor.matmul(
                                        scs[j][:, o + c0:o + c1],
                                        kT[c][base:base + 64, :],
                                        qT[m][base:base + 64,
                                              g * 256 + c0:g * 256 + c1],
                                        start=True, stop=True)
                                for j in jpair:
                                    if c in (2 * g, 2 * g + 1):      # diag
                                        d0 = o + (c - 2 * g) * 128
                                        nc.tensor.matmul(
                                            scs[j][:, d0:d0 + 128],
                                            mdiag[:, :], ident[:, :],
                                            start=False, stop=True,
                                            skip_group_check=True)
                                    if c + 8 in (2 * g, 2 * g + 1):  # far
                                        f0 = o + (c + 8 - 2 * g) * 128
                                        nc.tensor.matmul(
                                            scs[j][:, f0:f0 + 128],
                                            mfar[:, :], ident[:, :],
                                            start=False, stop=True,
                                            skip_group_check=True)
                                    if (c0, c1) != (0, 256):
                                        if c0 > 0:
                                            nc.gpsimd.memset(
                                                pts[j][:, o:o + c0], 0.0)
                                        if c1 < 256:
                                            nc.gpsimd.memset(
                                                pts[j][:, o + c1:o + 256], 0.0)
                                    pieces[j].append((pts[j], o))
                            lo = min(_qsel(c, g)[0] + 256 * p2
                                     for p2, c in enumerate(seg))
                            hi = max(_qsel(c, g)[1] + 256 * p2
                                     for p2, c in enumerate(seg))
                            for j in jpair:
                                nc.scalar.activation(
                                    pts[j][:, lo:hi], scs[j][:, lo:hi],
                                    AF.Exp, bias=0.0, scale=1.0)
                        for j in jpair:
                            u = j // 4
                            yt = ytpsp.tile([VSLOT, 256], F32, tag="yt")
                            for ci, c in enumerate(cs):
                                pt, o = pieces[j][ci]
                                nc.tensor.matmul(
                                    yt[:, :],
                                    v_sb[c][:, u * VSLOT:(u + 1) * VSLOT],
                                    pt[:, o:o + 256],
                                    start=(ci == 0), stop=(ci == len(cs) - 1))
                            nc.vector.tensor_copy(
                                yu_g[:, j * 256:(j + 1) * 256], yt[0:VSLOT, :])
                    # softmax denominators: gather, reciprocal, broadcast
                    sump = att2.tile([NH, 256], F32, tag="sump")
                    nc.sync.dma_start(sump[:, :], yu_g[64:65, :])
                    rec = att2.tile([NH, 256], F32R, tag="rec")
                    with nc.allow_low_precision(reason="fp32r recip broadcast"):
                        nc.vector.reciprocal(rec[:, :], sump[:, :])
                    recf = att2.tile([1, NH * 256], F32R, tag="recf")
                    nc.sync.dma_start(recf[:, :], rec[:, :])
                    ytg = [ytgp.tile([128, 256], BF16, tag=f"ytg{m}",
                                     name=f"ytg{g}_{m}")
                           for m in range(4)]
                    for j in range(NH):
                        rb = rbpsp.tile([64, 256], F32, tag="rb")
                        nc.tensor.matmul(rb[:, :], ones_r[:, :],
                                         recf[:, j * 256:(j + 1) * 256],
                                         start=True, stop=True)
                        m = j // 2
                        if j % 2 == 0:
                            nc.vector.tensor_mul(
                                ytg[m][0:64, :],
                                yu_g[0:64, j * 256:(j + 1) * 256], rb[:, :])
                        else:
                            yn = att2.tile([64, 256], BF16, tag="yn")
                            nc.vector.tensor_mul(
                                yn[:, :], yu_g[0:64, j * 256:(j + 1) * 256],
                                rb[:, :])
                            nc.sync.dma_start(ytg[m][64:128, :], yn[:, :])
                    # pipelined exchange for this query group
                    agin = dram2p.tile([4, 128, 256], BF16, tag="agin")
                    agout = dram2p.tile([8, 128, 256], BF16, tag="agout")
                    for m in range(4):
                        nc.sync.dma_start(agin[m], ytg[m][:, :])
                    if fake_collective:
                        nc.sync.dma_start(agout[0:4], agin[:, :, :])
                        nc.sync.dma_start(agout[4:8], agin[:, :, :])
                    else:
                        nc.gpsimd.collective_compute(
                            "AllGather", ALU.bypass,
                            replica_groups=[[0, 1], [2, 3], [4, 5], [6, 7]],
                            ins=[agin[:, :, :].opt()],
                            outs=[agout[:, :, :].opt()])
                    ygs = projp.tile([128, 8, 256], BF16, tag="ygs")
                    nc.sync.dma_start(
                        ygs[:, :, :],
                        agout[:, :, :].rearrange("c p n -> p c n"))
                    for i in (2 * g, 2 * g + 1):
                        io = (i - 2 * g) * 128
                        pr = prpsp.tile([128, 512], F32, tag="pr")
                        for cc in range(8):
                            nc.tensor.matmul(pr[:, :],
                                             ygs[:, cc, io:io + 128],
                                             wp_sb[:, cc],
                                             start=(cc == 0), stop=(cc == 7))
                        nc.vector.tensor_copy(o_all[:, i, :], pr[:, :])
                    nc.sync.dma_start(
                        y_d.ap()[g * 256:(g + 1) * 256, :].rearrange(
                            "(i p) n -> p i n", p=128),
                        o_all[:, 2 * g:2 * g + 2, :])

    nc.compile()
    return nc


def _prep_inputs(x, ve, cos, sin, wq, wk, wv, wproj, wgate):
    bf = ml_dtypes.bfloat16
    cosf = np.asarray(cos, np.float32).reshape(T, 32)
    sinf = np.asarray(sin, np.float32).reshape(T, 32)
    c2 = np.concatenate([cosf, cosf], axis=1).astype(bf)
    s2 = np.concatenate([sinf, -sinf], axis=1).astype(bf)
    x = np.asarray(x, np.float32)
    ve = np.asarray(ve, np.float32)
    wq = np.asarray(wq, np.float32)
    wk = np.asarray(wk, np.float32)
    wv = np.asarray(wv, np.float32)
    wproj = np.asarray(wproj, np.float32)
    wgate = np.asarray(wgate, np.float32)
    maps = []
    for core in range(8):
        b, hp = core // 2, core % 2
        maps.append({
            "x_bf": x[b].astype(bf),
            "ve_bf": ve[b][:, hp * 128:(hp + 1) * 128].astype(bf),
            "wq_bf": wq[:, hp * 512:(hp + 1) * 512].astype(bf),
            "wk_bf": wk[:, hp * 128:(hp + 1) * 128].astype(bf),
            "wv_bf": wv[:, hp * 128:(hp + 1) * 128].astype(bf),
            "wg_bf": wgate[:, hp * 2:(hp + 1) * 2].astype(bf),
            "wp_bf": wproj[:, hp * 512:(hp + 1) * 512].astype(bf),
            "c2_bf": c2,
            "s2_bf": s2,
            "ones_f": np.ones((1, 64), np.float32),
        })
    return maps


def kernel(x, ve, cos, sin, wq, wk, wv, wproj, wgate, window):
    assert int(window) == WINDOW
    if "nc" not in _CACHE:
        _CACHE["nc"] = build_program()
    nc = _CACHE["nc"]
    maps = _prep_inputs(x, ve, cos, sin, wq, wk, wv, wproj, wgate)
    res = run_bass_kernel_spmd(nc, maps, list(range(8))).results
    y = np.empty((B, T, C), np.float32)
    for core in range(8):
        b, hp = core // 2, core % 2
        y[b][:, hp * 512:(hp + 1) * 512] = res[core]["y_out"]
    return y
